# revision 8
# baseline (speedup 1.0000x reference)
"""Sparse GQA flex-attention with FP8-scale paged KV cache — TRN2, 8 NeuronCores.

Sharding: tensor-parallel by head. Core i gets q heads [4i, 4i+4), kv head i,
its kv-head slice of the paged caches, and the (replicated, host-transposed)
block mask. No collectives: each core computes its 4 heads' output; host
concatenates.

Per-core device pipeline:
  1. absmax(k), absmax(v) -> k_scale/v_scale (free-dim reduce + gpsimd
     partition_all_reduce, replicated per-partition).
  2. (only if slot_mapping overlaps cache_slots) quantize k,v by 1/scale and
     indirect-scatter into the cache tables.
  3. Indirect-gather the 3072 context rows from each cache table.
  4. K: TensorE-transpose ctx+new tiles into KT [d=128, 4096] bf16. The ctx
     dequant scale is NOT applied to K; it is folded into the exp() scale.
     V: dequant ctx rows by v_scale (DVE) into V [4096, 129] bf16 with a
     ones-column at 128 (yields softmax denominator for free during PV).
  5. Per head: per 128-key block: QK^T (scores^T in PSUM, queries on the free
     axis), exp via ACT (scale = SCALE or SCALE*k_scale for ctx blocks),
     mask-multiply on DVE, then PV matmuls accumulating [128q, 129] per
     q-block. Epilogue: divide by the ones-column sum, DMA out.
"""

import sys

for _p in ("/opt/trn_rl_repo",):
    if _p not in sys.path:
        sys.path.insert(0, _p)

import numpy as np

import concourse.bass as bass
import concourse.tile as tile
from concourse import bacc, bass_isa, mybir
from concourse.bass_utils import run_bass_kernel_spmd
from concourse.masks import make_identity

# Problem constants (hardcoded per spec)
H = 32
HKV = 8
D = 128
SCALE = D**-0.5
FP8_MAX = 448.0
EPS = 1e-8
PAGE = 256
NPAGES = 20
NSLOTS = NPAGES * PAGE  # 5120
SQ = 1024
CTX = 3072
SKV = CTX + SQ  # 4096
NCORES = 8
HL = H // NCORES  # 4 local q heads per core
P = 128
NCTX_T = CTX // P  # 24 context gather tiles
NNEW_T = SQ // P  # 8 new-token tiles
NKB = SKV // P  # 32 key blocks
NQB = SQ // P  # 8 query blocks

f32 = mybir.dt.float32
bf16 = mybir.dt.bfloat16
i32 = mybir.dt.int32
u8 = mybir.dt.uint8


def build_bass(with_scatter: bool) -> bacc.Bacc:
    nc = bacc.Bacc()

    q_d = nc.dram_tensor("q", [SQ, HL * D], f32, kind="ExternalInput")
    k_d = nc.dram_tensor("k", [SQ, D], f32, kind="ExternalInput")
    v_d = nc.dram_tensor("v", [SQ, D], f32, kind="ExternalInput")
    kc_d = nc.dram_tensor("kc", [NSLOTS, D], f32, kind="ExternalInput")
    vc_d = nc.dram_tensor("vc", [NSLOTS, D], f32, kind="ExternalInput")
    cs_d = nc.dram_tensor("cs", [P, NCTX_T], i32, kind="ExternalInput")
    mt_d = nc.dram_tensor("maskt", [SKV, SQ], u8, kind="ExternalInput")
    out_d = nc.dram_tensor("out", [SQ, HL * D], f32, kind="ExternalOutput")
    if with_scatter:
        sm_d = nc.dram_tensor("sm", [P, NNEW_T], i32, kind="ExternalInput")

    with tile.TileContext(nc) as tc:
        with (
            tc.tile_pool(name="const", bufs=1) as const,
            tc.tile_pool(name="persist", bufs=1) as persist,
            tc.tile_pool(name="stage", bufs=4) as stage,
            tc.tile_pool(name="mstage", bufs=3) as mstage,
            tc.tile_pool(name="pt", bufs=4) as ptp,
            tc.tile_pool(name="outp", bufs=4) as outp,
            tc.tile_pool(name="scores", bufs=2, space="PSUM") as scores_ps,
            tc.tile_pool(name="pv", bufs=1, space="PSUM") as pv_ps,
        ):
            ident = const.tile([P, P], f32)
            make_identity(nc, ident)

            cs_sb = const.tile([P, NCTX_T], i32)
            nc.sync.dma_start(out=cs_sb[:], in_=cs_d[:, :])

            # ---- load new k/v tiles; absmax stats ----
            knew = []
            vnew = []
            kabs = const.tile([P, 2 * NNEW_T], f32)
            for j in range(NNEW_T):
                kt_ = persist.tile([P, D], f32, tag=f"knew{j}")
                nc.sync.dma_start(out=kt_[:], in_=k_d[j * P : (j + 1) * P, :])
                knew.append(kt_)
                nc.vector.tensor_reduce(
                    out=kabs[:, j : j + 1],
                    in_=kt_[:],
                    axis=mybir.AxisListType.X,
                    op=mybir.AluOpType.max,
                    apply_absolute_value=True,
                )
                vt_ = persist.tile([P, D], f32, tag=f"vnew{j}")
                nc.sync.dma_start(out=vt_[:], in_=v_d[j * P : (j + 1) * P, :])
                vnew.append(vt_)
                nc.vector.tensor_reduce(
                    out=kabs[:, NNEW_T + j : NNEW_T + j + 1],
                    in_=vt_[:],
                    axis=mybir.AxisListType.X,
                    op=mybir.AluOpType.max,
                    apply_absolute_value=True,
                )

            kvmax = const.tile([P, 2], f32)
            nc.vector.tensor_reduce(
                out=kvmax[:, 0:1],
                in_=kabs[:, 0:NNEW_T],
                axis=mybir.AxisListType.X,
                op=mybir.AluOpType.max,
            )
            nc.vector.tensor_reduce(
                out=kvmax[:, 1:2],
                in_=kabs[:, NNEW_T : 2 * NNEW_T],
                axis=mybir.AxisListType.X,
                op=mybir.AluOpType.max,
            )
            kvmax_r = const.tile([P, 2], f32)
            nc.gpsimd.partition_all_reduce(
                out_ap=kvmax_r[:],
                in_ap=kvmax[:],
                channels=P,
                reduce_op=bass_isa.ReduceOp.max,
            )
            # k exp scale for ctx blocks: SCALE * max(absmax/448, EPS)
            kexp = const.tile([P, 1], f32)
            nc.vector.tensor_scalar(
                out=kexp[:],
                in0=kvmax_r[:, 0:1],
                scalar1=FP8_MAX * EPS,
                scalar2=SCALE / FP8_MAX,
                op0=mybir.AluOpType.max,
                op1=mybir.AluOpType.mult,
            )
            # v dequant scale: max(absmax/448, EPS)
            vdeq = const.tile([P, 1], f32)
            nc.vector.tensor_scalar(
                out=vdeq[:],
                in0=kvmax_r[:, 1:2],
                scalar1=FP8_MAX * EPS,
                scalar2=1.0 / FP8_MAX,
                op0=mybir.AluOpType.max,
                op1=mybir.AluOpType.mult,
            )

            if with_scatter:
                sm_sb = const.tile([P, NNEW_T], i32)
                nc.sync.dma_start(out=sm_sb[:], in_=sm_d[:, :])
                # inverse scales for quantization
                kscale = const.tile([P, 1], f32)
                nc.vector.tensor_scalar(
                    out=kscale[:],
                    in0=kvmax_r[:, 0:1],
                    scalar1=FP8_MAX * EPS,
                    scalar2=1.0 / FP8_MAX,
                    op0=mybir.AluOpType.max,
                    op1=mybir.AluOpType.mult,
                )
                kinv = const.tile([P, 1], f32)
                nc.vector.reciprocal(kinv[:], kscale[:])
                vinv = const.tile([P, 1], f32)
                nc.vector.reciprocal(vinv[:], vdeq[:])
                for j in range(NNEW_T):
                    kq = stage.tile([P, D], f32, tag="kq")
                    nc.vector.tensor_scalar_mul(kq[:], knew[j][:], kinv[:, 0:1])
                    nc.gpsimd.indirect_dma_start(
                        out=kc_d[:, :],
                        out_offset=bass.IndirectOffsetOnAxis(
                            ap=sm_sb[:, j : j + 1], axis=0
                        ),
                        in_=kq[:],
                        in_offset=None,
                    )
                    vq = stage.tile([P, D], f32, tag="vq")
                    nc.vector.tensor_scalar_mul(vq[:], vnew[j][:], vinv[:, 0:1])
                    nc.gpsimd.indirect_dma_start(
                        out=vc_d[:, :],
                        out_offset=bass.IndirectOffsetOnAxis(
                            ap=sm_sb[:, j : j + 1], axis=0
                        ),
                        in_=vq[:],
                        in_offset=None,
                    )
                # ensure all scatters land before any gather reads the tables
                tc.strict_bb_all_engine_barrier()

            # ---- persistent bf16 operands ----
            KT = persist.tile([P, SKV], bf16, tag="KT")  # [d, keys]
            V3 = persist.tile([P, NKB, D + 1], bf16, tag="V3")  # [tok, kb, d+1]
            QT = persist.tile([P, HL, SQ], bf16, tag="QT")  # [d, h, q]
            MB = persist.tile([P, NKB, SQ], bf16, tag="MB")  # [key, kb, q]

            nc.vector.memset(V3[:, :, D : D + 1], 1.0)

            # ---- gather ctx rows, build KT / V ----
            for t in range(NCTX_T):
                g = stage.tile([P, D], f32, tag="gk")
                nc.gpsimd.indirect_dma_start(
                    out=g[:],
                    out_offset=None,
                    in_=kc_d[:, :],
                    in_offset=bass.IndirectOffsetOnAxis(
                        ap=cs_sb[:, t : t + 1], axis=0
                    ),
                )
                tp = scores_ps.tile([P, P], f32, tag="scores")
                nc.tensor.transpose(out=tp[:], in_=g[:], identity=ident[:])
                nc.vector.tensor_copy(KT[:, t * P : (t + 1) * P], tp[:])

                g2 = stage.tile([P, D], f32, tag="gv")
                nc.gpsimd.indirect_dma_start(
                    out=g2[:],
                    out_offset=None,
                    in_=vc_d[:, :],
                    in_offset=bass.IndirectOffsetOnAxis(
                        ap=cs_sb[:, t : t + 1], axis=0
                    ),
                )
                nc.vector.tensor_scalar_mul(V3[:, t, 0:D], g2[:], vdeq[:, 0:1])

            for j in range(NNEW_T):
                tp = scores_ps.tile([P, P], f32, tag="scores")
                nc.tensor.transpose(out=tp[:], in_=knew[j][:], identity=ident[:])
                nc.vector.tensor_copy(
                    KT[:, (NCTX_T + j) * P : (NCTX_T + j + 1) * P], tp[:]
                )
                nc.vector.tensor_copy(V3[:, NCTX_T + j, 0:D], vnew[j][:])

            # ---- load+transpose Q ----
            for h in range(HL):
                for qb in range(NQB):
                    qs = stage.tile([P, D], f32, tag="qs")
                    nc.sync.dma_start(
                        out=qs[:],
                        in_=q_d[qb * P : (qb + 1) * P, h * D : (h + 1) * D],
                    )
                    tp = scores_ps.tile([P, P], f32, tag="scores")
                    nc.tensor.transpose(out=tp[:], in_=qs[:], identity=ident[:])
                    nc.vector.tensor_copy(QT[:, h, qb * P : (qb + 1) * P], tp[:])

            # ---- load mask^T, convert to bf16 ----
            for kb in range(NKB):
                ms = mstage.tile([P, SQ], u8, tag="ms")
                nc.sync.dma_start(out=ms[:], in_=mt_d[kb * P : (kb + 1) * P, :])
                nc.vector.tensor_copy(MB[:, kb, :], ms[:])

            # ---- main attention loop ----
            for h in range(HL):
                pv = pv_ps.tile([P, NQB, 256], f32, tag="pv")
                nc.vector.memset(pv[:], 0.0)
                for kb in range(NKB):
                    sc = scores_ps.tile([P, SQ], f32, tag="scores")
                    for half in range(2):
                        nc.tensor.matmul(
                            out=sc[:, half * 512 : (half + 1) * 512],
                            lhsT=KT[:, kb * P : (kb + 1) * P],
                            rhs=QT[:, h, half * 512 : (half + 1) * 512],
                            start=True,
                            stop=True,
                        )
                    pt = ptp.tile([P, SQ], bf16, tag="pt")
                    if kb < NCTX_T:
                        nc.scalar.activation(
                            out=pt[:],
                            in_=sc[:],
                            func=mybir.ActivationFunctionType.Exp,
                            scale=kexp[:, 0:1],
                        )
                    else:
                        nc.scalar.activation(
                            out=pt[:],
                            in_=sc[:],
                            func=mybir.ActivationFunctionType.Exp,
                            scale=SCALE,
                        )
                    nc.vector.tensor_mul(pt[:], pt[:], MB[:, kb, :])
                    for m in range(NQB):
                        nc.tensor.matmul(
                            out=pv[:, m, 0 : D + 1],
                            lhsT=pt[:, m * P : (m + 1) * P],
                            rhs=V3[:, kb, :],
                            start=False,
                            stop=(kb == NKB - 1),
                            skip_group_check=True,
                        )
                # epilogue: divide by denominator, store
                for m in range(NQB):
                    den = outp.tile([P, 1], f32, tag="den")
                    nc.vector.reciprocal(den[:], pv[:, m, D : D + 1])
                    ot = outp.tile([P, D], f32, tag="ot")
                    nc.vector.tensor_scalar_mul(ot[:], pv[:, m, 0:D], den[:, 0:1])
                    nc.sync.dma_start(
                        out=out_d[m * P : (m + 1) * P, h * D : (h + 1) * D],
                        in_=ot[:],
                    )

    return nc


_built: dict[bool, bacc.Bacc] = {}


def _get_built(with_scatter: bool) -> bacc.Bacc:
    if with_scatter not in _built:
        nc = build_bass(with_scatter)
        nc.compile()
        _built[with_scatter] = nc
    return _built[with_scatter]



def _ensure_ntff_hook():
    """Register the NTFF profile hook (ctypes into libaxon_pjrt.so) if the
    image's antenv lacks axon_hooks — enables trace=True exec_time_ns."""
    import types

    try:
        from antenv.axon_hooks import get_axon_ntff_profile_hook  # noqa: F401

        return
    except ImportError:
        pass
    import antenv

    mod = types.ModuleType("antenv.axon_hooks")
    mod._hook = None

    def set_axon_ntff_profile_hook(h):
        mod._hook = h

    def get_axon_ntff_profile_hook():
        return mod._hook

    mod.set_axon_ntff_profile_hook = set_axon_ntff_profile_hook
    mod.get_axon_ntff_profile_hook = get_axon_ntff_profile_hook
    sys.modules["antenv.axon_hooks"] = mod
    antenv.axon_hooks = mod
    try:
        sys.path.insert(0, "/root/.axon_site/trn_agent_boot")
        import trn_boot

        hook = trn_boot._ntff_profile_via_ctypes("/opt/axon/libaxon_pjrt.so")
        if hook is not None:
            set_axon_ntff_profile_hook(hook)
    except Exception:
        pass


LAST_EXEC_NS = None
LAST_RESULT = None


def _run(inputs: dict, trace: bool = False) -> np.ndarray:
    global LAST_EXEC_NS, LAST_RESULT
    q = np.asarray(inputs["q"], dtype=np.float32)
    k = np.asarray(inputs["k"], dtype=np.float32)
    v = np.asarray(inputs["v"], dtype=np.float32)
    k_cache = np.asarray(inputs["k_cache"], dtype=np.float32)
    v_cache = np.asarray(inputs["v_cache"], dtype=np.float32)
    slot_mapping = np.asarray(inputs["slot_mapping"], dtype=np.int32)
    cache_slots = np.asarray(inputs["cache_slots"], dtype=np.int32)
    block_mask = np.asarray(inputs["block_mask"])

    # scatter only observable through re-gather of overlapping slots
    with_scatter = bool(
        np.intersect1d(slot_mapping, cache_slots).size > 0
    )

    # host-side layout prep (metadata / replicated mask only)
    cs_perm = np.ascontiguousarray(
        cache_slots.reshape(NCTX_T, P).T
    )  # [P, NCTX_T]; cs_perm[p, t] = cache_slots[t*128 + p]
    maskt = np.ascontiguousarray(block_mask.T).astype(np.uint8)  # [SKV, SQ]
    if with_scatter:
        sm_perm = np.ascontiguousarray(slot_mapping.reshape(NNEW_T, P).T)

    in_maps = []
    for i in range(NCORES):
        m = {
            "q": np.ascontiguousarray(q[:, i * HL * D : (i + 1) * HL * D]),
            "k": np.ascontiguousarray(k[:, i * D : (i + 1) * D]),
            "v": np.ascontiguousarray(v[:, i * D : (i + 1) * D]),
            "kc": np.ascontiguousarray(k_cache[:, :, i, :]).reshape(NSLOTS, D),
            "vc": np.ascontiguousarray(v_cache[:, :, i, :]).reshape(NSLOTS, D),
            "cs": cs_perm,
            "maskt": maskt,
        }
        if with_scatter:
            m["sm"] = sm_perm
        in_maps.append(m)

    nc = _get_built(with_scatter)
    if trace:
        _ensure_ntff_hook()
    res = run_bass_kernel_spmd(
        nc, in_maps, core_ids=list(range(NCORES)), trace=trace
    )
    LAST_EXEC_NS = res.exec_time_ns
    LAST_RESULT = res
    out = np.concatenate([res.results[i]["out"] for i in range(NCORES)], axis=1)
    return np.ascontiguousarray(out, dtype=np.float32)


def kernel(**inputs) -> np.ndarray:
    return _run(inputs, trace=False)
